# revision 10
# baseline (speedup 1.0000x reference)
"""Trainium2 Bass kernel for DeepOdoModel (CNN feature extractor + GRU).

Single-core design (per-core NEFF launches serialize through this PJRT
path, so total device time is minimized by putting all 16 batch lanes on
one core):
- CNN/FC/gi stage in bf16 (moving operands bf16 -> 4x PE throughput),
  fp32 PSUM accumulation, frame order t-major so gi streams contiguously.
- gi projections spilled to DRAM (too big for SBUF at BL=16) and streamed
  back during the GRU in 32-step chunks (double buffered).
- GRU: biases/gi_rz folded into PSUM via identity-weight matmuls, r-gate
  matmuls emitted first so the sigmoid starts early; hidden state kept
  bf16 (matmul rhs) with fp32 gate math.
"""

import sys

if "/opt/trn_rl_repo" not in sys.path:
    sys.path.insert(0, "/opt/trn_rl_repo")

import numpy as np

B, T_FULL, L, C = 16, 512, 50, 7
H = 512
NCORES = 1
BL = B  # all batch lanes on one core


def build_nc(T=T_FULL, debug=False):
    import concourse.mybir as mybir
    import concourse.tile as tile
    from concourse import bacc
    from concourse.alu_op_type import AluOpType
    from concourse.tile_rust import add_dep_helper

    f32 = mybir.dt.float32
    bf16 = mybir.dt.bfloat16
    AF = mybir.ActivationFunctionType
    NF = BL * T
    F1 = 8
    F2 = 32
    F3 = 128
    n3 = NF // F3          # 64 blocks of 128 frames (8 t-steps x 16 b)
    n2 = F3 // F2
    n1 = F2 // F1
    TB = F3 // BL          # t-steps per block = 8
    NCH = 32               # GRU steps per gi stream chunk
    BPC = NCH // TB        # CNN blocks per gi chunk = 4

    nc = bacc.Bacc("TRN2", target_bir_lowering=False, debug=debug,
                   num_devices=NCORES)

    xim = nc.dram_tensor("xim", [77, NF * 40], bf16, kind="ExternalInput")
    h0t = nc.dram_tensor("h0t", [128, BL * 4], bf16, kind="ExternalInput")
    w1 = nc.dram_tensor("w1", [77, 128], bf16, kind="ExternalInput")
    w2t = nc.dram_tensor("w2t", [128, 9 * 256], bf16, kind="ExternalInput")
    wct = nc.dram_tensor("wct", [12 * 128, 512], bf16, kind="ExternalInput")
    wiht = nc.dram_tensor("wiht", [512, 1536], bf16, kind="ExternalInput")
    whht = nc.dram_tensor("whht", [512, 1536], bf16, kind="ExternalInput")
    ident = nc.dram_tensor("ident", [128, 128], bf16, kind="ExternalInput")
    bhhb = nc.dram_tensor("bhhb", [128, 4 * BL], bf16, kind="ExternalInput")
    gib = nc.dram_tensor("gib", [128, 12], f32, kind="ExternalInput")
    fc3w = nc.dram_tensor("fc3w", [128, 4], bf16, kind="ExternalInput")
    fc3b = nc.dram_tensor("fc3b", [1, 1], f32, kind="ExternalInput")
    giRZ = nc.dram_tensor("giRZ", [128, T * 8 * BL], bf16,
                          kind="ExternalInput")
    giN = nc.dram_tensor("giN", [128, T * 4 * BL], f32, kind="ExternalInput")
    out = nc.dram_tensor("out", [1, NF], f32, kind="ExternalOutput")

    GRZ = 8 * BL   # rz cols per step (j,b) = 128
    GN = 4 * BL    # n cols per step = 64
    HC = 4 * BL    # h cols per step (k,b) = 64

    with tile.TileContext(nc) as tc:
        with tc.tile_pool(name="weights", bufs=1) as wp:
            w1sb = wp.tile([77, 128], bf16)
            nc.sync.dma_start(out=w1sb, in_=w1.ap())
            w2sb = wp.tile([128, 9 * 256], bf16)
            nc.sync.dma_start(out=w2sb, in_=w2t.ap())
            wcsb = wp.tile([128, 12 * 512], bf16)
            for kt in range(12):
                nc.sync.dma_start(out=wcsb[:, kt * 512:(kt + 1) * 512],
                                  in_=wct.ap()[kt * 128:(kt + 1) * 128, :])
            wihsb = wp.tile([128, 4 * 1536], bf16)
            whhsb = wp.tile([128, 4 * 1536], bf16)
            for k in range(4):
                nc.sync.dma_start(out=wihsb[:, k * 1536:(k + 1) * 1536],
                                  in_=wiht.ap()[k * 128:(k + 1) * 128, :])
                nc.sync.dma_start(out=whhsb[:, k * 1536:(k + 1) * 1536],
                                  in_=whht.ap()[k * 128:(k + 1) * 128, :])
            idsb = wp.tile([128, 128], bf16)
            nc.sync.dma_start(out=idsb, in_=ident.ap())
            bhhsb = wp.tile([128, 4 * BL], bf16)
            nc.sync.dma_start(out=bhhsb, in_=bhhb.ap())
            gibsb = wp.tile([128, 12], f32)
            nc.sync.dma_start(out=gibsb, in_=gib.ap())
            fc3wsb = wp.tile([128, 4], bf16)
            nc.sync.dma_start(out=fc3wsb, in_=fc3w.ap())
            fc3bsb = wp.tile([1, 1], f32)
            nc.sync.dma_start(out=fc3bsb, in_=fc3b.ap())
            h0sb = wp.tile([128, HC], bf16)
            nc.sync.dma_start(out=h0sb, in_=h0t.ap())

            # persistent hidden states (bf16, feeds both GRU matmuls + head)
            hsT = wp.tile([128, T * HC], bf16)

            rz_out_dmas = []
            n_out_dmas = []

            # CNN + FC + gi emission as a generator of small units so it can
            # be interleaved into the GRU's latency gaps. PSUM tiles are
            # padded to a full 2KB bank so CNN start=True matmuls never share
            # a bank with in-flight GRU accumulation state.
            with tc.tile_pool(name="ps_cnn", bufs=4, space="PSUM") as psp, \
                 tc.tile_pool(name="xb", bufs=12) as xpool, \
                 tc.tile_pool(name="p1", bufs=2) as p1pool, \
                 tc.tile_pool(name="p2", bufs=2) as p2pool, \
                 tc.tile_pool(name="ft", bufs=2) as ftpool, \
                 tc.tile_pool(name="gst", bufs=2) as gspool, \
                 tc.tile_pool(name="psa", bufs=2, space="PSUM") as psap, \
                 tc.tile_pool(name="psb", bufs=2, space="PSUM") as psbp, \
                 tc.tile_pool(name="gin", bufs=3) as ginp, \
                 tc.tile_pool(name="gt", bufs=3) as gtp:

                def cnn_block(b3):
                    p2t = p2pool.tile([128, 2 * F3 * 6], bf16)
                    for b2 in range(n2):
                        p1t = p1pool.tile([128, F2 * 20], bf16)
                        for c1 in range(n1):
                            n0 = b3 * F3 + b2 * F2 + c1 * F1
                            x1 = xpool.tile([77, F1 * 40], bf16)
                            nc.sync.dma_start(
                                out=x1,
                                in_=xim.ap()[:, n0 * 40:(n0 + F1) * 40])
                            ps1 = psp.tile([128, 512], f32, tag="ps")
                            nc.tensor.matmul(
                                ps1[:, 0:F1 * 40], lhsT=w1sb[:], rhs=x1,
                                start=True, stop=True)
                            nc.vector.tensor_reduce(
                                out=p1t[:, c1 * F1 * 20:(c1 + 1) * F1 * 20],
                                in_=ps1[:, 0:F1 * 40].rearrange(
                                    "p (a two) -> p a two", two=2),
                                axis=mybir.AxisListType.X, op=AluOpType.max)
                            yield
                        p1v = p1t.rearrange("p (n l) -> p n l", l=20)
                        for m in range(2):
                            ps2 = psp.tile([128, 512], f32, tag="ps")
                            for k in range(9):
                                nc.tensor.matmul(
                                    ps2[:, 0:F2 * 12],
                                    lhsT=w2sb[:, k * 256 + m * 128:
                                              k * 256 + m * 128 + 128],
                                    rhs=p1v[:, :, k:k + 12],
                                    start=(k == 0), stop=(k == 8))
                            nc.vector.tensor_reduce(
                                out=p2t[:, m * F3 * 6 + b2 * F2 * 6:
                                        m * F3 * 6 + (b2 + 1) * F2 * 6],
                                in_=ps2[:, 0:F2 * 12].rearrange(
                                    "p (a two) -> p a two", two=2),
                                axis=mybir.AxisListType.X, op=AluOpType.max)
                            yield
                    ft = ftpool.tile([128, 4 * F3], bf16)
                    p2v = p2t.rearrange("p (c n l) -> p c n l", c=2, l=6)
                    for m4 in range(4):
                        ps3 = psp.tile([128, 512], f32, tag="ps")
                        for kt in range(12):
                            p_, cm = kt // 2, kt % 2
                            nc.tensor.matmul(
                                ps3[:, 0:F3],
                                lhsT=wcsb[:, kt * 512 + m4 * 128:
                                          kt * 512 + m4 * 128 + 128],
                                rhs=p2v[:, cm, :, p_:p_ + 1],
                                start=(kt == 0), stop=(kt == 11))
                        nc.scalar.copy(ft[:, m4 * F3:(m4 + 1) * F3],
                                       ps3[:, 0:F3])
                        yield
                    gRZt = gspool.tile([128, TB * GRZ], bf16, tag="grz")
                    gNt = gspool.tile([128, TB * GN], f32, tag="gn")
                    gRZv = gRZt.rearrange("p (tt c) -> p tt c", c=GRZ)
                    gNv = gNt.rearrange("p (tt c) -> p tt c", c=GN)
                    for j in range(12):
                        ps4 = psp.tile([128, 512], f32, tag="ps")
                        for k in range(4):
                            nc.tensor.matmul(
                                ps4[:, 0:F3],
                                lhsT=wihsb[:, k * 1536 + j * 128:
                                           k * 1536 + j * 128 + 128],
                                rhs=ft[:, k * F3:(k + 1) * F3],
                                start=(k == 0), stop=(k == 3))
                        src = ps4[:, 0:F3].rearrange("p (tt b) -> p tt b",
                                                     b=BL)
                        if j < 8:
                            dst = gRZv[:, :, j * BL:(j + 1) * BL]
                        else:
                            dst = gNv[:, :, (j - 8) * BL:(j - 7) * BL]
                        nc.scalar.activation(dst, src, AF.Identity,
                                             bias=gibsb[:, j:j + 1])
                        yield
                    d1 = nc.sync.dma_start(
                        out=giRZ.ap()[:, b3 * TB * GRZ:(b3 + 1) * TB * GRZ],
                        in_=gRZt)
                    d2 = nc.sync.dma_start(
                        out=giN.ap()[:, b3 * TB * GN:(b3 + 1) * TB * GN],
                        in_=gNt)
                    rz_out_dmas.append(d1)
                    n_out_dmas.append(d2)
                    yield

                def cnn_all():
                    for b3 in range(n3):
                        yield from cnn_block(b3)

                UPB = n1 * n2 + 2 * n2 + 4 + 12 + 1   # units per block = 35
                gen = cnn_all()
                emitted = 0

                def emit_until(target):
                    nonlocal emitted
                    while emitted < target:
                        try:
                            next(gen)
                        except StopIteration:
                            emitted = n3 * UPB
                            return
                        emitted += 1

                WARM_BLOCKS = 8
                emit_until(WARM_BLOCKS * UPB)
                total_units = n3 * UPB
                rest = total_units - WARM_BLOCKS * UPB
                RAMP_STEPS = T - 64   # finish CNN emission by step T-64

                def fetch_chunk(ch):
                    grz_t = ginp.tile([128, NCH * GRZ], bf16, tag="rz")
                    gn_t = ginp.tile([128, NCH * GN], f32, tag="n")
                    di1 = nc.sync.dma_start(
                        out=grz_t,
                        in_=giRZ.ap()[:, ch * NCH * GRZ:
                                      (ch + 1) * NCH * GRZ])
                    di2 = nc.sync.dma_start(
                        out=gn_t,
                        in_=giN.ap()[:, ch * NCH * GN:(ch + 1) * NCH * GN])
                    blk = ch * BPC + BPC - 1
                    add_dep_helper(di1.ins, rz_out_dmas[blk].ins,
                                   reason="gi dram raw")
                    add_dep_helper(di2.ins, n_out_dmas[blk].ins,
                                   reason="gi dram raw")
                    return grz_t, gn_t

                prev_mm = None
                cur = fetch_chunk(0)
                nxt = fetch_chunk(1)
                for ch in range(T // NCH):
                    grz_t, gn_t = cur
                    for tl in range(NCH):
                        t = ch * NCH + tl
                        if t < RAMP_STEPS:
                            emit_until(WARM_BLOCKS * UPB +
                                       (t + 1) * rest // RAMP_STEPS)
                        hprev = h0sb if t == 0 else hsT[:, (t - 1) * HC:
                                                        t * HC]
                        psgA = psap.tile([128, 512], f32)
                        psgB = psbp.tile([128, 512], f32)
                        # PSUM accumulation: one start=True per region;
                        # strict PE emission order via dep chain.
                        mms = []
                        mms.append(nc.tensor.matmul(
                            psgA[:, 0:GRZ], lhsT=idsb[:],
                            rhs=grz_t[:, tl * GRZ:(tl + 1) * GRZ],
                            start=True, stop=False, skip_group_check=True))
                        for m in range(8):
                            for k in range(4):
                                mms.append(nc.tensor.matmul(
                                    psgA[:, m * BL:(m + 1) * BL],
                                    lhsT=whhsb[:, k * 1536 + m * 128:
                                               k * 1536 + m * 128 + 128],
                                    rhs=hprev[:, k * BL:(k + 1) * BL],
                                    start=False, stop=(k == 3),
                                    skip_group_check=True))
                        mms.append(nc.tensor.matmul(
                            psgB[:, 0:GN], lhsT=idsb[:], rhs=bhhsb[:],
                            start=True, stop=False, skip_group_check=True))
                        for m in range(4):
                            for k in range(4):
                                mms.append(nc.tensor.matmul(
                                    psgB[:, m * BL:(m + 1) * BL],
                                    lhsT=whhsb[:, k * 1536 + (m + 8) * 128:
                                               k * 1536 + (m + 8) * 128 + 128],
                                    rhs=hprev[:, k * BL:(k + 1) * BL],
                                    start=False, stop=(k == 3),
                                    skip_group_check=True))
                        for mm in mms:
                            if prev_mm is not None:
                                add_dep_helper(mm.ins, prev_mm.ins,
                                               reason="psum group order")
                            prev_mm = mm
                        rt = gtp.tile([128, 4 * BL], f32)
                        nc.scalar.activation(rt, psgA[:, 0:4 * BL],
                                             AF.Sigmoid)
                        zt = gtp.tile([128, 4 * BL], f32)
                        nc.scalar.activation(zt, psgA[:, 4 * BL:8 * BL],
                                             AF.Sigmoid)
                        tmp = gtp.tile([128, GN], f32)
                        nc.vector.tensor_tensor(out=tmp, in0=psgB[:, 0:GN],
                                                in1=rt[:],
                                                op=AluOpType.mult)
                        npre = gtp.tile([128, GN], f32)
                        nc.vector.tensor_tensor(
                            out=npre, in0=tmp,
                            in1=gn_t[:, tl * GN:(tl + 1) * GN],
                            op=AluOpType.add)
                        nt = gtp.tile([128, GN], f32)
                        nc.scalar.activation(nt, npre, AF.Tanh)
                        d = gtp.tile([128, GN], f32)
                        nc.vector.tensor_tensor(out=d, in0=hprev, in1=nt,
                                                op=AluOpType.subtract)
                        e = gtp.tile([128, GN], f32)
                        nc.vector.tensor_tensor(out=e, in0=d, in1=zt,
                                                op=AluOpType.mult)
                        nc.vector.tensor_tensor(
                            out=hsT[:, t * HC:(t + 1) * HC], in0=e, in1=nt,
                            op=AluOpType.add)
                    if ch + 2 < T // NCH:
                        cur, nxt = nxt, fetch_chunk(ch + 2)
                    else:
                        cur = nxt

            # ---------------- output head --------------------------------
            with tc.tile_pool(name="pso", bufs=2, space="PSUM") as psop, \
                 tc.tile_pool(name="ot", bufs=1) as otp:
                osb = otp.tile([1, NF], f32)
                hs4 = hsT.rearrange("p (tt k b) -> p tt k b", k=4, b=BL)
                CH = 32
                for ch in range(T // CH):
                    pso = psop.tile([1, CH * BL], f32)
                    for k in range(4):
                        nc.tensor.matmul(
                            pso[:],
                            lhsT=fc3wsb[:, k:k + 1],
                            rhs=hs4[:, ch * CH:(ch + 1) * CH, k, :],
                            start=(k == 0), stop=(k == 3))
                    nc.scalar.activation(
                        osb[:, ch * CH * BL:(ch + 1) * CH * BL],
                        pso[:], AF.Identity, bias=fc3bsb[:, 0:1])
                nc.sync.dma_start(out=out.ap(), in_=osb)

    nc.compile()
    return nc


def prep_inputs(inputs, T=T_FULL):
    import ml_dtypes
    bf = ml_dtypes.bfloat16
    f = np.float32
    conv1_w = inputs["conv1_w"].astype(f)
    conv1_b = inputs["conv1_b"].astype(f)
    conv2_w = inputs["conv2_w"].astype(f)
    conv2_b = inputs["conv2_b"].astype(f)
    fc1_w = inputs["fc1_w"].astype(f)
    fc1_b = inputs["fc1_b"].astype(f)
    fc2_w = inputs["fc2_w"].astype(f)
    fc2_b = inputs["fc2_b"].astype(f)
    wih = inputs["gru_wih"].astype(f)
    whh = inputs["gru_whh"].astype(f)
    bih = inputs["gru_bih"].astype(f)
    bhh = inputs["gru_bhh"].astype(f)
    fc3_w = inputs["fc3_w"].astype(f)
    fc3_b = inputs["fc3_b"].astype(f)

    w1 = np.ascontiguousarray(
        conv1_w.transpose(1, 2, 0).reshape(77, 128)).astype(bf)
    w2tt = np.ascontiguousarray(
        conv2_w.transpose(1, 2, 0).reshape(128, 9 * 256)).astype(bf)

    Wc = fc2_w @ fc1_w
    b2_eff = conv2_b + np.einsum("oik,i->o", conv2_w, conv1_b)
    b2_flat = np.repeat(b2_eff, 6)
    bc_eff = fc2_w @ fc1_b + fc2_b + Wc @ b2_flat

    WcT = Wc.T
    wct = np.empty((12 * 128, 512), f)
    for p in range(6):
        for cm in range(2):
            kt = p * 2 + cm
            rows = 6 * (cm * 128 + np.arange(128)) + p
            wct[kt * 128:(kt + 1) * 128] = WcT[rows]
    wct = wct.astype(bf)

    gi_bias = bih + wih @ bc_eff
    gi_bias[:1024] += bhh[:1024]
    gib = np.ascontiguousarray(gi_bias.reshape(12, 128).T).astype(f)
    bhhn = bhh[1024:].reshape(4, 128).T            # [128, 4]
    bhhb = np.repeat(bhhn[:, :, None], BL, axis=2).reshape(128, 4 * BL)
    bhhb = np.ascontiguousarray(bhhb).astype(bf)
    fc3wt = np.ascontiguousarray(fc3_w[0].reshape(4, 128).T).astype(bf)

    wihT = np.ascontiguousarray(wih.T).astype(bf)
    whhT = np.ascontiguousarray(whh.T).astype(bf)
    ident = np.eye(128, dtype=f).astype(bf)

    phone = inputs["phone_data"].astype(f)         # [B, T, L, C]
    h0 = inputs["h0"].astype(f)

    NF = BL * T
    xt = np.ascontiguousarray(
        phone.transpose(1, 0, 2, 3).reshape(NF, L, C))  # t-major frames
    sw = np.lib.stride_tricks.sliding_window_view(xt, 40, axis=1)
    # sw: [NF, 11, C, 40] with sw[f, k, c, j] = xt[f, k + j, c]
    xim = np.ascontiguousarray(
        sw.transpose(2, 1, 0, 3)).reshape(77, NF * 40).astype(bf)

    h0tt = np.ascontiguousarray(
        h0.reshape(BL, 4, 128).transpose(2, 1, 0).reshape(128, 4 * BL))
    h0tt = h0tt.astype(bf)

    giRZ = np.zeros((128, T * 8 * BL), bf)
    giN = np.zeros((128, T * 4 * BL), f)

    in_map = {
        "xim": xim, "h0t": h0tt, "w1": w1, "w2t": w2tt, "wct": wct,
        "wiht": wihT, "whht": whhT, "ident": ident, "bhhb": bhhb,
        "gib": gib, "fc3w": fc3wt,
        "fc3b": fc3_b.reshape(1, 1).astype(f),
        "giRZ": giRZ, "giN": giN,
    }
    return [in_map]


def assemble_output(results, T=T_FULL):
    o = results[0]["out"].reshape(T, BL)   # col = t*BL + b
    full = np.ascontiguousarray(o.T).reshape(BL, T, 1).astype(np.float32)
    return full


_NC_CACHE = {}


def kernel(**inputs):
    from concourse import bass_utils

    if "nc" not in _NC_CACHE:
        _NC_CACHE["nc"] = build_nc()
    nc = _NC_CACHE["nc"]
    in_maps = prep_inputs(inputs)
    res = bass_utils.run_bass_kernel_spmd(nc, in_maps, core_ids=[0])
    return assemble_output(res.results)


# revision 12
# speedup vs baseline: 1.8121x; 1.8121x over previous
"""Trainium2 Bass kernel for DeepOdoModel (CNN feature extractor + GRU).

Single-core design (per-core NEFF launches serialize through this PJRT
path, so total device time is minimized by putting all 16 batch lanes on
one core):
- CNN/FC/gi stage in bf16 (moving operands bf16 -> 4x PE throughput),
  fp32 PSUM accumulation, frame order t-major so gi streams contiguously.
- gi projections spilled to DRAM (too big for SBUF at BL=16) and streamed
  back during the GRU in 32-step chunks (double buffered).
- GRU: biases/gi_rz folded into PSUM via identity-weight matmuls, r-gate
  matmuls emitted first so the sigmoid starts early; hidden state kept
  bf16 (matmul rhs) with fp32 gate math.
"""

import sys

if "/opt/trn_rl_repo" not in sys.path:
    sys.path.insert(0, "/opt/trn_rl_repo")

import numpy as np

B, T_FULL, L, C = 16, 512, 50, 7
H = 512
NCORES = 1
BL = B  # all batch lanes on one core


def build_nc(T=T_FULL, debug=False):
    import concourse.mybir as mybir
    import concourse.tile as tile
    from concourse import bacc
    from concourse.alu_op_type import AluOpType
    from concourse.tile_rust import add_dep_helper

    f32 = mybir.dt.float32
    bf16 = mybir.dt.bfloat16
    AF = mybir.ActivationFunctionType
    NF = BL * T
    F1 = 8
    F2 = 32
    F3 = 128
    n3 = NF // F3          # 64 blocks of 128 frames (8 t-steps x 16 b)
    n2 = F3 // F2
    n1 = F2 // F1
    TB = F3 // BL          # t-steps per block = 8
    NCH = 32               # GRU steps per gi stream chunk
    BPC = NCH // TB        # CNN blocks per gi chunk = 4

    nc = bacc.Bacc("TRN2", target_bir_lowering=False, debug=debug,
                   num_devices=NCORES)

    xim = nc.dram_tensor("xim", [77, NF * 40], bf16, kind="ExternalInput")
    h0t = nc.dram_tensor("h0t", [128, BL * 4], bf16, kind="ExternalInput")
    w1 = nc.dram_tensor("w1", [77, 128], bf16, kind="ExternalInput")
    w2t = nc.dram_tensor("w2t", [128, 9 * 256], bf16, kind="ExternalInput")
    wct = nc.dram_tensor("wct", [12 * 128, 512], bf16, kind="ExternalInput")
    wiht = nc.dram_tensor("wiht", [512, 1536], bf16, kind="ExternalInput")
    whht = nc.dram_tensor("whht", [512, 1536], bf16, kind="ExternalInput")
    ident = nc.dram_tensor("ident", [128, 128], bf16, kind="ExternalInput")
    bhhb = nc.dram_tensor("bhhb", [128, 4 * BL], bf16, kind="ExternalInput")
    gib = nc.dram_tensor("gib", [128, 12], f32, kind="ExternalInput")
    fc3w = nc.dram_tensor("fc3w", [128, 4], bf16, kind="ExternalInput")
    fc3b = nc.dram_tensor("fc3b", [1, 1], f32, kind="ExternalInput")
    giRZ = nc.dram_tensor("giRZ", [128, T * 8 * BL], bf16,
                          kind="ExternalInput")
    giN = nc.dram_tensor("giN", [128, T * 4 * BL], f32, kind="ExternalInput")
    out = nc.dram_tensor("out", [1, NF], f32, kind="ExternalOutput")

    GRZ = 8 * BL   # rz cols per step (j,b) = 128
    GN = 4 * BL    # n cols per step = 64
    HC = 4 * BL    # h cols per step (k,b) = 64

    with tile.TileContext(nc) as tc:
        with tc.tile_pool(name="weights", bufs=1) as wp:
            w1sb = wp.tile([77, 128], bf16)
            nc.sync.dma_start(out=w1sb, in_=w1.ap())
            w2sb = wp.tile([128, 9 * 256], bf16)
            nc.sync.dma_start(out=w2sb, in_=w2t.ap())
            wcsb = wp.tile([128, 12 * 512], bf16)
            for kt in range(12):
                nc.sync.dma_start(out=wcsb[:, kt * 512:(kt + 1) * 512],
                                  in_=wct.ap()[kt * 128:(kt + 1) * 128, :])
            wihsb = wp.tile([128, 4 * 1536], bf16)
            whhsb = wp.tile([128, 4 * 1536], bf16)
            for k in range(4):
                nc.sync.dma_start(out=wihsb[:, k * 1536:(k + 1) * 1536],
                                  in_=wiht.ap()[k * 128:(k + 1) * 128, :])
                nc.sync.dma_start(out=whhsb[:, k * 1536:(k + 1) * 1536],
                                  in_=whht.ap()[k * 128:(k + 1) * 128, :])
            idsb = wp.tile([128, 128], bf16)
            nc.sync.dma_start(out=idsb, in_=ident.ap())
            bhhsb = wp.tile([128, 4 * BL], bf16)
            nc.sync.dma_start(out=bhhsb, in_=bhhb.ap())
            gibsb = wp.tile([128, 12], f32)
            nc.sync.dma_start(out=gibsb, in_=gib.ap())
            fc3wsb = wp.tile([128, 4], bf16)
            nc.sync.dma_start(out=fc3wsb, in_=fc3w.ap())
            fc3bsb = wp.tile([1, 1], f32)
            nc.sync.dma_start(out=fc3bsb, in_=fc3b.ap())
            h0sb = wp.tile([128, HC], bf16)
            nc.sync.dma_start(out=h0sb, in_=h0t.ap())

            # persistent hidden states (bf16, feeds both GRU matmuls + head)
            hsT = wp.tile([128, T * HC], bf16)

            rz_out_dmas = []
            n_out_dmas = []

            # CNN + FC + gi emission as a generator of small units so it can
            # be interleaved into the GRU's latency gaps. PSUM tiles are
            # padded to a full 2KB bank so CNN start=True matmuls never share
            # a bank with in-flight GRU accumulation state.
            with tc.tile_pool(name="ps_cnn", bufs=4, space="PSUM") as psp, \
                 tc.tile_pool(name="xb", bufs=12) as xpool, \
                 tc.tile_pool(name="p1", bufs=2) as p1pool, \
                 tc.tile_pool(name="p2", bufs=2) as p2pool, \
                 tc.tile_pool(name="ft", bufs=2) as ftpool, \
                 tc.tile_pool(name="gst", bufs=2) as gspool, \
                 tc.tile_pool(name="psa", bufs=2, space="PSUM") as psap, \
                 tc.tile_pool(name="psb", bufs=2, space="PSUM") as psbp, \
                 tc.tile_pool(name="gin", bufs=3) as ginp, \
                 tc.tile_pool(name="gt", bufs=3) as gtp:

                def cnn_block(b3):
                    p2t = p2pool.tile([128, 2 * F3 * 6], bf16)
                    for b2 in range(n2):
                        p1t = p1pool.tile([128, F2 * 20], bf16)
                        for c1 in range(n1):
                            n0 = b3 * F3 + b2 * F2 + c1 * F1
                            x1 = xpool.tile([77, F1 * 40], bf16)
                            nc.sync.dma_start(
                                out=x1,
                                in_=xim.ap()[:, n0 * 40:(n0 + F1) * 40])
                            ps1 = psp.tile([128, 512], f32, tag="ps")
                            nc.tensor.matmul(
                                ps1[:, 0:F1 * 40], lhsT=w1sb[:], rhs=x1,
                                start=True, stop=True)
                            nc.vector.tensor_reduce(
                                out=p1t[:, c1 * F1 * 20:(c1 + 1) * F1 * 20],
                                in_=ps1[:, 0:F1 * 40].rearrange(
                                    "p (a two) -> p a two", two=2),
                                axis=mybir.AxisListType.X, op=AluOpType.max)
                            yield
                        p1v = p1t.rearrange("p (n l) -> p n l", l=20)
                        for m in range(2):
                            ps2 = psp.tile([128, 512], f32, tag="ps")
                            for k in range(9):
                                nc.tensor.matmul(
                                    ps2[:, 0:F2 * 12],
                                    lhsT=w2sb[:, k * 256 + m * 128:
                                              k * 256 + m * 128 + 128],
                                    rhs=p1v[:, :, k:k + 12],
                                    start=(k == 0), stop=(k == 8))
                            nc.vector.tensor_reduce(
                                out=p2t[:, m * F3 * 6 + b2 * F2 * 6:
                                        m * F3 * 6 + (b2 + 1) * F2 * 6],
                                in_=ps2[:, 0:F2 * 12].rearrange(
                                    "p (a two) -> p a two", two=2),
                                axis=mybir.AxisListType.X, op=AluOpType.max)
                            yield
                    ft = ftpool.tile([128, 4 * F3], bf16)
                    p2v = p2t.rearrange("p (c n l) -> p c n l", c=2, l=6)
                    for m4 in range(4):
                        ps3 = psp.tile([128, 512], f32, tag="ps")
                        for kt in range(12):
                            p_, cm = kt // 2, kt % 2
                            nc.tensor.matmul(
                                ps3[:, 0:F3],
                                lhsT=wcsb[:, kt * 512 + m4 * 128:
                                          kt * 512 + m4 * 128 + 128],
                                rhs=p2v[:, cm, :, p_:p_ + 1],
                                start=(kt == 0), stop=(kt == 11))
                        nc.scalar.copy(ft[:, m4 * F3:(m4 + 1) * F3],
                                       ps3[:, 0:F3])
                        yield
                    gRZt = gspool.tile([128, TB * GRZ], bf16, tag="grz")
                    gNt = gspool.tile([128, TB * GN], f32, tag="gn")
                    gRZv = gRZt.rearrange("p (tt c) -> p tt c", c=GRZ)
                    gNv = gNt.rearrange("p (tt c) -> p tt c", c=GN)
                    for j in range(12):
                        ps4 = psp.tile([128, 512], f32, tag="ps")
                        for k in range(4):
                            nc.tensor.matmul(
                                ps4[:, 0:F3],
                                lhsT=wihsb[:, k * 1536 + j * 128:
                                           k * 1536 + j * 128 + 128],
                                rhs=ft[:, k * F3:(k + 1) * F3],
                                start=(k == 0), stop=(k == 3))
                        src = ps4[:, 0:F3].rearrange("p (tt b) -> p tt b",
                                                     b=BL)
                        if j < 8:
                            dst = gRZv[:, :, j * BL:(j + 1) * BL]
                        else:
                            dst = gNv[:, :, (j - 8) * BL:(j - 7) * BL]
                        nc.scalar.activation(dst, src, AF.Identity,
                                             bias=gibsb[:, j:j + 1])
                        yield
                    d1 = nc.sync.dma_start(
                        out=giRZ.ap()[:, b3 * TB * GRZ:(b3 + 1) * TB * GRZ],
                        in_=gRZt)
                    d2 = nc.sync.dma_start(
                        out=giN.ap()[:, b3 * TB * GN:(b3 + 1) * TB * GN],
                        in_=gNt)
                    rz_out_dmas.append(d1)
                    n_out_dmas.append(d2)
                    yield

                def cnn_all():
                    for b3 in range(n3):
                        yield from cnn_block(b3)

                UPB = n1 * n2 + 2 * n2 + 4 + 12 + 1   # units per block = 35
                gen = cnn_all()
                emitted = 0

                def emit_until(target):
                    nonlocal emitted
                    while emitted < target:
                        try:
                            next(gen)
                        except StopIteration:
                            emitted = n3 * UPB
                            return
                        emitted += 1

                WARM_BLOCKS = 8
                emit_until(WARM_BLOCKS * UPB)
                total_units = n3 * UPB
                rest = total_units - WARM_BLOCKS * UPB
                RAMP_STEPS = T - 64   # finish CNN emission by step T-64

                def fetch_chunk(ch):
                    grz_t = ginp.tile([128, NCH * GRZ], bf16, tag="rz")
                    gn_t = ginp.tile([128, NCH * GN], f32, tag="n")
                    di1 = nc.sync.dma_start(
                        out=grz_t,
                        in_=giRZ.ap()[:, ch * NCH * GRZ:
                                      (ch + 1) * NCH * GRZ])
                    di2 = nc.sync.dma_start(
                        out=gn_t,
                        in_=giN.ap()[:, ch * NCH * GN:(ch + 1) * NCH * GN])
                    blk = ch * BPC + BPC - 1
                    add_dep_helper(di1.ins, rz_out_dmas[blk].ins,
                                   reason="gi dram raw")
                    add_dep_helper(di2.ins, n_out_dmas[blk].ins,
                                   reason="gi dram raw")
                    return grz_t, gn_t

                prev_mm = None
                cur = fetch_chunk(0)
                nxt = fetch_chunk(1)
                for ch in range(T // NCH):
                    grz_t, gn_t = cur
                    for tl in range(NCH):
                        t = ch * NCH + tl
                        if t < RAMP_STEPS:
                            emit_until(WARM_BLOCKS * UPB +
                                       (t + 1) * rest // RAMP_STEPS)
                        hprev = h0sb if t == 0 else hsT[:, (t - 1) * HC:
                                                        t * HC]
                        psgA = psap.tile([128, 512], f32)
                        psgB = psbp.tile([128, 512], f32)
                        # PSUM accumulation: one start=True per region;
                        # strict PE emission order via dep chain.
                        mms = []
                        mms.append(nc.tensor.matmul(
                            psgA[:, 0:GRZ], lhsT=idsb[:],
                            rhs=grz_t[:, tl * GRZ:(tl + 1) * GRZ],
                            start=True, stop=False, skip_group_check=True))
                        for m in range(8):
                            for k in range(4):
                                mms.append(nc.tensor.matmul(
                                    psgA[:, m * BL:(m + 1) * BL],
                                    lhsT=whhsb[:, k * 1536 + m * 128:
                                               k * 1536 + m * 128 + 128],
                                    rhs=hprev[:, k * BL:(k + 1) * BL],
                                    start=False, stop=(k == 3),
                                    skip_group_check=True))
                        mms.append(nc.tensor.matmul(
                            psgB[:, 0:GN], lhsT=idsb[:], rhs=bhhsb[:],
                            start=True, stop=False, skip_group_check=True))
                        for m in range(4):
                            for k in range(4):
                                mms.append(nc.tensor.matmul(
                                    psgB[:, m * BL:(m + 1) * BL],
                                    lhsT=whhsb[:, k * 1536 + (m + 8) * 128:
                                               k * 1536 + (m + 8) * 128 + 128],
                                    rhs=hprev[:, k * BL:(k + 1) * BL],
                                    start=False, stop=(k == 3),
                                    skip_group_check=True))
                        for mm in mms:
                            if prev_mm is not None:
                                add_dep_helper(mm.ins, prev_mm.ins,
                                               reason="psum group order")
                            prev_mm = mm
                        rt = gtp.tile([128, 4 * BL], f32)
                        nc.scalar.activation(rt, psgA[:, 0:4 * BL],
                                             AF.Sigmoid)
                        zt = gtp.tile([128, 4 * BL], f32)
                        nc.scalar.activation(zt, psgA[:, 4 * BL:8 * BL],
                                             AF.Sigmoid)
                        tmp = gtp.tile([128, GN], f32)
                        nc.vector.tensor_tensor(out=tmp, in0=psgB[:, 0:GN],
                                                in1=rt[:],
                                                op=AluOpType.mult)
                        npre = gtp.tile([128, GN], f32)
                        nc.vector.tensor_tensor(
                            out=npre, in0=tmp,
                            in1=gn_t[:, tl * GN:(tl + 1) * GN],
                            op=AluOpType.add)
                        nt = gtp.tile([128, GN], f32)
                        nc.scalar.activation(nt, npre, AF.Tanh)
                        d = gtp.tile([128, GN], f32)
                        nc.vector.tensor_tensor(out=d, in0=hprev, in1=nt,
                                                op=AluOpType.subtract)
                        e = gtp.tile([128, GN], f32)
                        nc.vector.tensor_tensor(out=e, in0=d, in1=zt,
                                                op=AluOpType.mult)
                        nc.vector.tensor_tensor(
                            out=hsT[:, t * HC:(t + 1) * HC], in0=e, in1=nt,
                            op=AluOpType.add)
                    if ch + 2 < T // NCH:
                        cur, nxt = nxt, fetch_chunk(ch + 2)
                    else:
                        cur = nxt

            # ---------------- output head --------------------------------
            with tc.tile_pool(name="pso", bufs=2, space="PSUM") as psop, \
                 tc.tile_pool(name="ot", bufs=1) as otp:
                osb = otp.tile([1, NF], f32)
                hs4 = hsT.rearrange("p (tt k b) -> p tt k b", k=4, b=BL)
                CH = 32
                for ch in range(T // CH):
                    pso = psop.tile([1, CH * BL], f32)
                    for k in range(4):
                        nc.tensor.matmul(
                            pso[:],
                            lhsT=fc3wsb[:, k:k + 1],
                            rhs=hs4[:, ch * CH:(ch + 1) * CH, k, :],
                            start=(k == 0), stop=(k == 3))
                    nc.scalar.activation(
                        osb[:, ch * CH * BL:(ch + 1) * CH * BL],
                        pso[:], AF.Identity, bias=fc3bsb[:, 0:1])
                nc.sync.dma_start(out=out.ap(), in_=osb)

    nc.compile()
    return nc


def prep_inputs(inputs, T=T_FULL):
    import ml_dtypes
    bf = ml_dtypes.bfloat16
    f = np.float32
    conv1_w = inputs["conv1_w"].astype(f)
    conv1_b = inputs["conv1_b"].astype(f)
    conv2_w = inputs["conv2_w"].astype(f)
    conv2_b = inputs["conv2_b"].astype(f)
    fc1_w = inputs["fc1_w"].astype(f)
    fc1_b = inputs["fc1_b"].astype(f)
    fc2_w = inputs["fc2_w"].astype(f)
    fc2_b = inputs["fc2_b"].astype(f)
    wih = inputs["gru_wih"].astype(f)
    whh = inputs["gru_whh"].astype(f)
    bih = inputs["gru_bih"].astype(f)
    bhh = inputs["gru_bhh"].astype(f)
    fc3_w = inputs["fc3_w"].astype(f)
    fc3_b = inputs["fc3_b"].astype(f)

    w1 = np.ascontiguousarray(
        conv1_w.transpose(1, 2, 0).reshape(77, 128)).astype(bf)
    w2tt = np.ascontiguousarray(
        conv2_w.transpose(1, 2, 0).reshape(128, 9 * 256)).astype(bf)

    Wc = fc2_w @ fc1_w
    b2_eff = conv2_b + np.einsum("oik,i->o", conv2_w, conv1_b)
    b2_flat = np.repeat(b2_eff, 6)
    bc_eff = fc2_w @ fc1_b + fc2_b + Wc @ b2_flat

    WcT = Wc.T
    wct = np.empty((12 * 128, 512), f)
    for p in range(6):
        for cm in range(2):
            kt = p * 2 + cm
            rows = 6 * (cm * 128 + np.arange(128)) + p
            wct[kt * 128:(kt + 1) * 128] = WcT[rows]
    wct = wct.astype(bf)

    gi_bias = bih + wih @ bc_eff
    gi_bias[:1024] += bhh[:1024]
    gib = np.ascontiguousarray(gi_bias.reshape(12, 128).T).astype(f)
    bhhn = bhh[1024:].reshape(4, 128).T            # [128, 4]
    bhhb = np.repeat(bhhn[:, :, None], BL, axis=2).reshape(128, 4 * BL)
    bhhb = np.ascontiguousarray(bhhb).astype(bf)
    fc3wt = np.ascontiguousarray(fc3_w[0].reshape(4, 128).T).astype(bf)

    wihT = np.ascontiguousarray(wih.T).astype(bf)
    whhT = np.ascontiguousarray(whh.T).astype(bf)
    ident = np.eye(128, dtype=f).astype(bf)

    phone = inputs["phone_data"].astype(f)         # [B, T, L, C]
    h0 = inputs["h0"].astype(f)

    NF = BL * T
    xt = np.ascontiguousarray(
        phone.transpose(1, 0, 2, 3).reshape(NF, L, C))  # t-major frames
    sw = np.lib.stride_tricks.sliding_window_view(xt, 40, axis=1)
    # sw: [NF, 11, C, 40] with sw[f, k, c, j] = xt[f, k + j, c]
    xim = np.ascontiguousarray(
        sw.transpose(2, 1, 0, 3)).reshape(77, NF * 40).astype(bf)

    h0tt = np.ascontiguousarray(
        h0.reshape(BL, 4, 128).transpose(2, 1, 0).reshape(128, 4 * BL))
    h0tt = h0tt.astype(bf)

    giRZ = np.zeros((128, T * 8 * BL), bf)
    giN = np.zeros((128, T * 4 * BL), f)

    in_map = {
        "xim": xim, "h0t": h0tt, "w1": w1, "w2t": w2tt, "wct": wct,
        "wiht": wihT, "whht": whhT, "ident": ident, "bhhb": bhhb,
        "gib": gib, "fc3w": fc3wt,
        "fc3b": fc3_b.reshape(1, 1).astype(f),
        "giRZ": giRZ, "giN": giN,
    }
    return [in_map]


def assemble_output(results, T=T_FULL):
    o = results[0]["out"].reshape(T, BL)   # col = t*BL + b
    full = np.ascontiguousarray(o.T).reshape(BL, T, 1).astype(np.float32)
    return full


_NC_CACHE = {}


def kernel(**inputs):
    from concourse import bass_utils

    if "nc" not in _NC_CACHE:
        _NC_CACHE["nc"] = build_nc()
    nc = _NC_CACHE["nc"]
    in_maps = prep_inputs(inputs)
    res = bass_utils.run_bass_kernel_spmd(nc, in_maps, core_ids=[0])
    return assemble_output(res.results)
